# revision 5
# baseline (speedup 1.0000x reference)
"""DeepSeek-V3 MoE block on 8 trn2 NeuronCores.

Expert-parallel sparse MoE, fp16 datapath:
  - host computes routing (top-k indices AND combine weights) in fp32 numpy;
    the device never sees the gate -- it receives gathered tokens, fp16
    weights, and a per-token fp32 scale applied at the down projection
  - experts sorted by token count into 4 tiers of 8; core c slot k holds the
    (8k+c)-th busiest expert; slot capacity = exact max count of its tier so
    every core runs the identical program (SPMD) with zero weight duplication
  - all matmul operands fp16 (1 cycle/row on the PE, half the HBM traffic of
    fp32); accumulation in fp32 PSUM; outputs shipped back fp16
  - weights/tokens are host-pretransposed to p-major [128, k, n] so every
    load is a single fully-contiguous DMA
  - shared expert (intermediate dim sharded 8-way) is computed FIRST: its
    small DMA prefix starts the PE early and its compute covers the expert
    weight stream
  - host sums the 8 shared-expert partials and scatter-adds the routed rows
"""

import os
import sys

sys.path.insert(0, "/opt/trn_rl_repo")

import numpy as np

import concourse.bacc as bacc
import concourse.bass as bass
import concourse.mybir as mybir
import concourse.tile as tile
from concourse.bass_utils import run_bass_kernel_spmd

F32 = mybir.dt.float32
F16 = mybir.dt.float16
AF = mybir.ActivationFunctionType
ALU = mybir.AluOpType
AX = mybir.AxisListType

T, H, I, IS, E = 1024, 1024, 512, 2048, 32
G, TOPK_GROUP, TOP_K = 8, 4, 8
SCALE = 2.5
NCORES = 8
S = E // NCORES          # expert slots per core
ISH = IS // NCORES       # shared-expert intermediate shard
P128 = 128
HT = H // P128           # 8 h-tiles
IT = I // P128           # 4 i-tiles
IST = ISH // P128        # 2 shared i-tiles

LAST_RESULTS = None      # BassKernelResults of the most recent run


def _install_ntff_hook():
    """Provide antenv.axon_hooks + the ctypes NTFF profile hook when the
    container image lacks them (needed only for trace=True)."""
    import contextlib
    import ctypes
    import types

    try:
        from antenv.axon_hooks import get_axon_ntff_profile_hook  # noqa: F401
        return True
    except ImportError:
        pass
    try:
        import antenv
        so_path = "/opt/axon/libaxon_pjrt.so"
        lib = ctypes.CDLL(so_path)
        if not hasattr(lib, "axon_start_nrt_profile"):
            return False
        lib.axon_start_nrt_profile.argtypes = [
            ctypes.POINTER(ctypes.c_int64), ctypes.c_size_t]
        lib.axon_start_nrt_profile.restype = ctypes.c_int64
        lib.axon_stop_nrt_profile.argtypes = [ctypes.c_char_p]
        lib.axon_stop_nrt_profile.restype = ctypes.c_int64

        @contextlib.contextmanager
        def _hook(output_dir, device_ids):
            import jax
            jax.devices()
            if device_ids:
                ids = (ctypes.c_int64 * len(device_ids))(*device_ids)
                rc = lib.axon_start_nrt_profile(ids, len(device_ids))
            else:
                rc = lib.axon_start_nrt_profile(None, 0)
            if rc != 0:
                raise RuntimeError(f"axon_start_nrt_profile rc={rc}")
            try:
                yield
            finally:
                n = lib.axon_stop_nrt_profile(str(output_dir).encode())
                print(f"ntff profile: {n} file(s) -> {output_dir}",
                      file=sys.stderr)

        mod = types.ModuleType("antenv.axon_hooks")
        _state = {"hook": _hook}
        mod.set_axon_ntff_profile_hook = lambda h: _state.__setitem__("hook", h)
        mod.get_axon_ntff_profile_hook = lambda: _state["hook"]
        sys.modules["antenv.axon_hooks"] = mod
        antenv.axon_hooks = mod
        return True
    except Exception:
        return False


def _host_routing(x, gate_w, e_bias):
    """fp32 numpy mirror of reference._routing_combine.

    Returns (emask [T,E] bool, combine [T,E] fp32)."""
    logits = x.astype(np.float32) @ gate_w.T.astype(np.float32)
    scores = 1.0 / (1.0 + np.exp(-logits))
    swb = scores + e_bias[None, :]
    swb_g = swb.reshape(T, G, E // G)
    gs = np.sort(swb_g, axis=-1)[..., -2:].sum(-1)          # top-2 sum per group
    thr4 = np.sort(gs, axis=-1)[:, -TOPK_GROUP][:, None]
    gmask = (gs >= thr4).astype(np.float32)
    smask = np.repeat(gmask, E // G, axis=-1)
    masked = swb * smask
    thr8 = np.sort(masked, axis=-1)[:, -TOP_K][:, None]
    emask = masked >= thr8
    sc = scores * emask
    combine = sc / (sc.sum(-1, keepdims=True) + 1e-20) * SCALE
    return emask, combine.astype(np.float32)


def _chunks(p, limit=512):
    """Split width p into chunks <= limit."""
    out = []
    o = 0
    while o < p:
        w = min(limit, p - o)
        out.append((o, w))
        o += w
    return out


def _pmajor(a, p=P128):
    """[k*128, n] -> contiguous [128, k, n] (partition-major for 1-shot DMA)."""
    k = a.shape[0] // p
    return np.ascontiguousarray(a.reshape(k, p, a.shape[1]).transpose(1, 0, 2))


def _build_program(P):
    """Emit the SPMD Bass program for slot capacities P (list of S ints)."""
    nc = bacc.Bacc(target_bir_lowering=False, debug=False)

    # ---- DRAM parameters (per-core data arrives via in_maps) ----
    xe_d = [nc.dram_tensor(f"xe{k}", [P128, HT, P[k]], F16, kind="ExternalInput")
            for k in range(S)]
    wg_d = [nc.dram_tensor(f"wg{k}", [P128, HT, I], F16, kind="ExternalInput")
            for k in range(S)]
    wu_d = [nc.dram_tensor(f"wu{k}", [P128, HT, I], F16, kind="ExternalInput")
            for k in range(S)]
    wd_d = [nc.dram_tensor(f"wd{k}", [P128, IT, H], F16, kind="ExternalInput")
            for k in range(S)]
    nck = [(P[k] + P128 - 1) // P128 for k in range(S)]
    sc_d = [nc.dram_tensor(f"sc{k}", [P128, nck[k]], F32, kind="ExternalInput")
            for k in range(S)]
    xt_d = nc.dram_tensor("xt", [P128, HT, T], F16, kind="ExternalInput")
    wsg_d = nc.dram_tensor("wsg", [P128, HT, ISH], F16, kind="ExternalInput")
    wsu_d = nc.dram_tensor("wsu", [P128, HT, ISH], F16, kind="ExternalInput")
    wsd_d = nc.dram_tensor("wsd", [P128, IST, H], F16, kind="ExternalInput")
    ro_d = [nc.dram_tensor(f"ro{k}", [P[k], H], F16, kind="ExternalOutput")
            for k in range(S)]
    so_d = nc.dram_tensor("so", [T, H], F16, kind="ExternalOutput")

    PMAX = max(P)

    with tile.TileContext(nc) as tc:
        with (
            tc.tile_pool(name="const", bufs=1) as cpool,
            tc.tile_pool(name="x", bufs=2) as xpool,
            tc.tile_pool(name="w", bufs=2) as wpool,
            tc.tile_pool(name="acts", bufs=2) as apool,
            tc.tile_pool(name="stage", bufs=3) as stpool,
            tc.tile_pool(name="ps", bufs=2, space="PSUM") as ps,
        ):
            # ---- load order == consumption order ----
            # shared expert inputs first (smallest prefix that starts the PE)
            xt_t = cpool.tile([P128, HT, T], F16, tag="xt")
            nc.sync.dma_start(out=xt_t[:], in_=xt_d[:])
            wsg_t = cpool.tile([P128, HT, ISH], F16, tag="wsg")
            nc.sync.dma_start(out=wsg_t[:], in_=wsg_d[:])
            wsu_t = cpool.tile([P128, HT, ISH], F16, tag="wsu")
            nc.sync.dma_start(out=wsu_t[:], in_=wsu_d[:])
            wsd_t = cpool.tile([P128, IST, H], F16, tag="wsd")
            nc.sync.dma_start(out=wsd_t[:], in_=wsd_d[:])
            sc_t = []
            for k in range(S):
                t = cpool.tile([P128, nck[k]], F32, tag=f"sc{k}")
                nc.sync.dma_start(out=t[:], in_=sc_d[k][:])
                sc_t.append(t)

            # slot input streams (rings sized for 2 slots in flight)
            xe_t, wg_t, wu_t, wd_t = {}, {}, {}, {}
            def _load_slot(k):
                xe_t[k] = xpool.tile([P128, HT, P[k]], F16, tag="xe",
                                     name=f"xe_t{k}")
                nc.sync.dma_start(out=xe_t[k][:], in_=xe_d[k][:])
                wg_t[k] = wpool.tile([P128, HT, I], F16, tag="wg", bufs=2,
                                     name=f"wg_t{k}")
                nc.sync.dma_start(out=wg_t[k][:], in_=wg_d[k][:])
                wu_t[k] = wpool.tile([P128, HT, I], F16, tag="wu", bufs=2,
                                     name=f"wu_t{k}")
                nc.sync.dma_start(out=wu_t[k][:], in_=wu_d[k][:])
                wd_t[k] = wpool.tile([P128, IT, H], F16, tag="wd", bufs=2,
                                     name=f"wd_t{k}")
                nc.sync.dma_start(out=wd_t[k][:], in_=wd_d[k][:])

            _load_slot(0)
            _load_slot(1)

            # ---- shared expert (intermediate shard ISH=256) ----
            acts_s = [apool.tile([P128, T], F16, tag="acts_s", bufs=2,
                                 name=f"acts_s{ii}") for ii in range(IST)]
            for (mo, mw) in _chunks(T):
                for ii in range(IST):
                    h1 = ps.tile([P128, 512], F32, tag="h1")
                    h2 = ps.tile([P128, 512], F32, tag="h2")
                    for h in range(HT):
                        nc.tensor.matmul(
                            h1[:, :mw], lhsT=wsg_t[:, h, ii * P128:(ii + 1) * P128],
                            rhs=xt_t[:, h, mo:mo + mw],
                            start=(h == 0), stop=(h == HT - 1))
                    for h in range(HT):
                        nc.tensor.matmul(
                            h2[:, :mw], lhsT=wsu_t[:, h, ii * P128:(ii + 1) * P128],
                            rhs=xt_t[:, h, mo:mo + mw],
                            start=(h == 0), stop=(h == HT - 1))
                    sl = stpool.tile([P128, 512], F32, tag="silu", bufs=3,
                                     name="sl")
                    nc.scalar.activation(sl[:, :mw], h1[:, :mw], AF.Silu)
                    nc.vector.tensor_mul(acts_s[ii][:, mo:mo + mw],
                                         sl[:, :mw], h2[:, :mw])

            for cc in range(T // P128):
                for hh in range(2):
                    dps = ps.tile([P128, H // 2], F32, tag="dps")
                    for ii in range(IST):
                        nc.tensor.matmul(
                            dps[:],
                            lhsT=acts_s[ii][:, cc * P128:(cc + 1) * P128],
                            rhs=wsd_t[:, ii, hh * (H // 2):(hh + 1) * (H // 2)],
                            start=(ii == 0), stop=(ii == IST - 1))
                    ost = stpool.tile([P128, H // 2], F16, tag="ost")
                    nc.vector.tensor_copy(ost[:], dps[:])
                    nc.gpsimd.dma_start(
                        out=so_d[cc * P128:(cc + 1) * P128,
                                 hh * (H // 2):(hh + 1) * (H // 2)],
                        in_=ost[:])

            # ---- routed expert slots (descending capacity) ----
            for k in range(S):
                if k + 2 < S:
                    _load_slot(k + 2)

                acts = [apool.tile([P128, P[k]], F16, tag="acts", bufs=8,
                                   name=f"acts{ii}") for ii in range(IT)]
                for (mo, mw) in _chunks(P[k]):
                    for ii in range(IT):
                        h1 = ps.tile([P128, 512], F32, tag="h1")
                        h2 = ps.tile([P128, 512], F32, tag="h2")
                        for h in range(HT):
                            nc.tensor.matmul(
                                h1[:, :mw],
                                lhsT=wg_t[k][:, h, ii * P128:(ii + 1) * P128],
                                rhs=xe_t[k][:, h, mo:mo + mw],
                                start=(h == 0), stop=(h == HT - 1))
                        for h in range(HT):
                            nc.tensor.matmul(
                                h2[:, :mw],
                                lhsT=wu_t[k][:, h, ii * P128:(ii + 1) * P128],
                                rhs=xe_t[k][:, h, mo:mo + mw],
                                start=(h == 0), stop=(h == HT - 1))
                        sl = stpool.tile([P128, 512], F32, tag="silu", bufs=3,
                                         name="sl")
                        nc.scalar.activation(sl[:, :mw], h1[:, :mw], AF.Silu)
                        nc.vector.tensor_mul(acts[ii][:, mo:mo + mw],
                                             sl[:, :mw], h2[:, :mw])

                for cc in range(nck[k]):
                    pp = min(P128, P[k] - cc * P128)
                    for hh in range(2):
                        dps = ps.tile([P128, H // 2], F32, tag="dps")
                        for ii in range(IT):
                            nc.tensor.matmul(
                                dps[:pp, :],
                                lhsT=acts[ii][:, cc * P128:cc * P128 + pp],
                                rhs=wd_t[k][:, ii,
                                            hh * (H // 2):(hh + 1) * (H // 2)],
                                start=(ii == 0), stop=(ii == IT - 1))
                        ost = stpool.tile([P128, H // 2], F16, tag="ost")
                        nc.vector.tensor_scalar_mul(
                            ost[:pp, :], dps[:pp, :], sc_t[k][:pp, cc:cc + 1])
                        nc.gpsimd.dma_start(
                            out=ro_d[k][cc * P128:cc * P128 + pp,
                                        hh * (H // 2):(hh + 1) * (H // 2)],
                            in_=ost[:pp, :])

    nc.compile()
    return nc


def _prepare(inputs):
    """Host-side dispatch prep: returns (in_maps, P, slot_expert, tok_lists)."""
    x = np.ascontiguousarray(inputs["hidden_states"], dtype=np.float32)
    gate_w = np.asarray(inputs["gate_w"], dtype=np.float32)
    e_bias = np.asarray(inputs["e_bias"], dtype=np.float32)
    w_gate = np.asarray(inputs["w_gate"], dtype=np.float32)
    w_up = np.asarray(inputs["w_up"], dtype=np.float32)
    w_down = np.asarray(inputs["w_down"], dtype=np.float32)
    ws_gate = np.asarray(inputs["ws_gate"], dtype=np.float32)
    ws_up = np.asarray(inputs["ws_up"], dtype=np.float32)
    ws_down = np.asarray(inputs["ws_down"], dtype=np.float32)

    # ---- dispatch metadata ----
    emask, combine = _host_routing(x, gate_w, e_bias)
    counts = emask.sum(0).astype(np.int64)
    order = np.argsort(-counts, kind="stable")
    tok_lists = [np.nonzero(emask[:, e])[0] for e in range(E)]
    # slot k of every core serves tier k (descending size); capacity = exact
    # max count of the tier (moving dims need no padding in fp16)
    tiers = [[int(order[k * NCORES + c]) for c in range(NCORES)]
             for k in range(S)]
    P = [max(8, max(int(counts[e]) for e in tier)) for tier in tiers]

    x16 = x.astype(np.float16)
    xt = _pmajor(np.ascontiguousarray(x16.T))          # [128, HT, T]
    in_maps = []
    slot_expert = np.zeros((NCORES, S), dtype=np.int64)
    wg16 = w_gate.astype(np.float16)
    wu16 = w_up.astype(np.float16)
    wd16 = w_down.astype(np.float16)
    for c in range(NCORES):
        m = {"xt": xt,
             "wsg": _pmajor(ws_gate[:, c * ISH:(c + 1) * ISH].astype(np.float16)),
             "wsu": _pmajor(ws_up[:, c * ISH:(c + 1) * ISH].astype(np.float16)),
             "wsd": _pmajor(ws_down[c * ISH:(c + 1) * ISH, :].astype(np.float16))}
        for k in range(S):
            e = tiers[k][c]
            slot_expert[c, k] = e
            toks = tok_lists[e]
            n = len(toks)
            n_c = (P[k] + P128 - 1) // P128
            xe = np.zeros((P128, HT, P[k]), dtype=np.float16)
            xe[:, :, :n] = _pmajor(np.ascontiguousarray(x16[toks].T))
            scv = np.zeros(n_c * P128, dtype=np.float32)
            scv[:n] = combine[toks, e]
            m[f"xe{k}"] = xe
            m[f"wg{k}"] = _pmajor(wg16[e])
            m[f"wu{k}"] = _pmajor(wu16[e])
            m[f"wd{k}"] = _pmajor(wd16[e])
            m[f"sc{k}"] = np.ascontiguousarray(scv.reshape(n_c, P128).T)
        in_maps.append(m)

    return in_maps, P, slot_expert, tok_lists


def _recombine(results, slot_expert, tok_lists):
    out = np.zeros((T, H), dtype=np.float32)
    for c in range(NCORES):
        out += results[c]["so"].astype(np.float32)
    for c in range(NCORES):
        for k in range(S):
            e = slot_expert[c, k]
            toks = tok_lists[e]
            out[toks] += results[c][f"ro{k}"][:len(toks)].astype(np.float32)
    return out


def kernel(**inputs):
    global LAST_RESULTS
    in_maps, P, slot_expert, tok_lists = _prepare(inputs)
    nc = _build_program(P)
    trace = bool(int(os.environ.get("KERNEL_TRACE", "0")))
    if trace:
        trace = _install_ntff_hook()
    LAST_RESULTS = run_bass_kernel_spmd(
        nc, in_maps, list(range(NCORES)), trace=trace)
    results = LAST_RESULTS.results
    return _recombine(results, slot_expert, tok_lists)


# revision 6
# speedup vs baseline: 1.0600x; 1.0600x over previous
"""DeepSeek-V3 MoE block on 8 trn2 NeuronCores.

Expert-parallel sparse MoE, fp16 datapath:
  - host computes routing (top-k indices AND combine weights) in fp32 numpy;
    the device never sees the gate -- it receives gathered tokens, fp16
    weights, and a per-token fp32 scale applied at the down projection
  - experts sorted by token count into 4 tiers of 8; core c slot k holds the
    (8k+c)-th busiest expert; slot capacity = exact max count of its tier so
    every core runs the identical program (SPMD) with zero weight duplication
  - all matmul operands fp16 (1 cycle/row on the PE, half the HBM traffic of
    fp32); accumulation in fp32 PSUM; outputs shipped back fp16
  - weights/tokens are host-pretransposed to partition-major layouts so every
    load is a handful of fully-contiguous DMAs
  - compute order: smallest slot first (tiny DMA prefix starts the PE early),
    then the shared expert (intermediate dim sharded 8-way; its compute
    covers the big slots' weight stream), then remaining slots descending
  - host sums the 8 shared-expert partials and scatter-adds the routed rows
"""

import os
import sys

sys.path.insert(0, "/opt/trn_rl_repo")

import numpy as np

import concourse.bacc as bacc
import concourse.bass as bass
import concourse.mybir as mybir
import concourse.tile as tile
from concourse.bass_utils import run_bass_kernel_spmd

F32 = mybir.dt.float32
F16 = mybir.dt.float16
AF = mybir.ActivationFunctionType
ALU = mybir.AluOpType
AX = mybir.AxisListType

T, H, I, IS, E = 1024, 1024, 512, 2048, 32
G, TOPK_GROUP, TOP_K = 8, 4, 8
SCALE = 2.5
NCORES = 8
S = E // NCORES          # expert slots per core
ISH = IS // NCORES       # shared-expert intermediate shard
P128 = 128
HT = H // P128           # 8 h-tiles
IT = I // P128           # 4 i-tiles
IST = ISH // P128        # 2 shared i-tiles

LAST_RESULTS = None      # BassKernelResults of the most recent run


def _install_ntff_hook():
    """Provide antenv.axon_hooks + the ctypes NTFF profile hook when the
    container image lacks them (needed only for trace=True)."""
    import contextlib
    import ctypes
    import types

    try:
        from antenv.axon_hooks import get_axon_ntff_profile_hook  # noqa: F401
        return True
    except ImportError:
        pass
    try:
        import antenv
        so_path = "/opt/axon/libaxon_pjrt.so"
        lib = ctypes.CDLL(so_path)
        if not hasattr(lib, "axon_start_nrt_profile"):
            return False
        lib.axon_start_nrt_profile.argtypes = [
            ctypes.POINTER(ctypes.c_int64), ctypes.c_size_t]
        lib.axon_start_nrt_profile.restype = ctypes.c_int64
        lib.axon_stop_nrt_profile.argtypes = [ctypes.c_char_p]
        lib.axon_stop_nrt_profile.restype = ctypes.c_int64

        @contextlib.contextmanager
        def _hook(output_dir, device_ids):
            import jax
            jax.devices()
            if device_ids:
                ids = (ctypes.c_int64 * len(device_ids))(*device_ids)
                rc = lib.axon_start_nrt_profile(ids, len(device_ids))
            else:
                rc = lib.axon_start_nrt_profile(None, 0)
            if rc != 0:
                raise RuntimeError(f"axon_start_nrt_profile rc={rc}")
            try:
                yield
            finally:
                n = lib.axon_stop_nrt_profile(str(output_dir).encode())
                print(f"ntff profile: {n} file(s) -> {output_dir}",
                      file=sys.stderr)

        mod = types.ModuleType("antenv.axon_hooks")
        _state = {"hook": _hook}
        mod.set_axon_ntff_profile_hook = lambda h: _state.__setitem__("hook", h)
        mod.get_axon_ntff_profile_hook = lambda: _state["hook"]
        sys.modules["antenv.axon_hooks"] = mod
        antenv.axon_hooks = mod
        return True
    except Exception:
        return False


def _host_routing(x, gate_w, e_bias):
    """fp32 numpy mirror of reference._routing_combine.

    Returns (emask [T,E] bool, combine [T,E] fp32)."""
    logits = x.astype(np.float32) @ gate_w.T.astype(np.float32)
    scores = 1.0 / (1.0 + np.exp(-logits))
    swb = scores + e_bias[None, :]
    swb_g = swb.reshape(T, G, E // G)
    gs = np.sort(swb_g, axis=-1)[..., -2:].sum(-1)          # top-2 sum per group
    thr4 = np.sort(gs, axis=-1)[:, -TOPK_GROUP][:, None]
    gmask = (gs >= thr4).astype(np.float32)
    smask = np.repeat(gmask, E // G, axis=-1)
    masked = swb * smask
    thr8 = np.sort(masked, axis=-1)[:, -TOP_K][:, None]
    emask = masked >= thr8
    sc = scores * emask
    combine = sc / (sc.sum(-1, keepdims=True) + 1e-20) * SCALE
    return emask, combine.astype(np.float32)


def _chunks(p, limit=512):
    """Split width p into chunks <= limit."""
    out = []
    o = 0
    while o < p:
        w = min(limit, p - o)
        out.append((o, w))
        o += w
    return out


def _pmajor(a, p=P128):
    """[k*128, n] -> contiguous [128, k, n] (partition-major for 1-shot DMA)."""
    k = a.shape[0] // p
    return np.ascontiguousarray(a.reshape(k, p, a.shape[1]).transpose(1, 0, 2))


def _iimajor(a):
    """[HT*128, IT*128] weight -> contiguous [128, IT, HT, 128] so each
    [:, ii] slice is one fully-contiguous DMA (per-ii streaming)."""
    m = a.shape[1] // P128
    b = a.reshape(HT, P128, m, P128).transpose(1, 2, 0, 3)
    return np.ascontiguousarray(b)


def _build_program(P):
    """Emit the SPMD Bass program for slot capacities P (list of S ints)."""
    nc = bacc.Bacc(target_bir_lowering=False, debug=False)

    # ---- DRAM parameters (per-core data arrives via in_maps) ----
    xe_d = [nc.dram_tensor(f"xe{k}", [P128, HT, P[k]], F16, kind="ExternalInput")
            for k in range(S)]
    wg_d = [nc.dram_tensor(f"wg{k}", [P128, IT, HT, P128], F16,
                           kind="ExternalInput") for k in range(S)]
    wu_d = [nc.dram_tensor(f"wu{k}", [P128, IT, HT, P128], F16,
                           kind="ExternalInput") for k in range(S)]
    wd_d = [nc.dram_tensor(f"wd{k}", [P128, IT, H], F16, kind="ExternalInput")
            for k in range(S)]
    nck = [(P[k] + P128 - 1) // P128 for k in range(S)]
    sc_d = [nc.dram_tensor(f"sc{k}", [P128, nck[k]], F32, kind="ExternalInput")
            for k in range(S)]
    xt_d = nc.dram_tensor("xt", [P128, HT, T], F16, kind="ExternalInput")
    wsg_d = nc.dram_tensor("wsg", [P128, IST, HT, P128], F16,
                           kind="ExternalInput")
    wsu_d = nc.dram_tensor("wsu", [P128, IST, HT, P128], F16,
                           kind="ExternalInput")
    wsd_d = nc.dram_tensor("wsd", [P128, IST, H], F16, kind="ExternalInput")
    ro_d = [nc.dram_tensor(f"ro{k}", [P[k], H], F16, kind="ExternalOutput")
            for k in range(S)]
    so_d = nc.dram_tensor("so", [T, H], F16, kind="ExternalOutput")

    with tile.TileContext(nc) as tc:
        with (
            tc.tile_pool(name="const", bufs=1) as cpool,
            tc.tile_pool(name="x", bufs=3) as xpool,
            tc.tile_pool(name="w", bufs=3) as wpool,
            tc.tile_pool(name="acts", bufs=2) as apool,
            tc.tile_pool(name="stage", bufs=3) as stpool,
            tc.tile_pool(name="ps", bufs=2, space="PSUM") as ps,
        ):
            # ---- loads, in consumption order ----
            sc_t = []
            for k in range(S):
                t = cpool.tile([P128, nck[k]], F32, tag=f"sc{k}")
                nc.sync.dma_start(out=t[:], in_=sc_d[k][:])
                sc_t.append(t)

            xe_t, wg_t, wu_t, wd_t = {}, {}, {}, {}

            def _load_slot(k):
                xe_t[k] = xpool.tile([P128, HT, P[k]], F16, tag="xe",
                                     name=f"xe_t{k}")
                nc.sync.dma_start(out=xe_t[k][:], in_=xe_d[k][:])
                wg_t[k] = wpool.tile([P128, IT, HT, P128], F16, tag="wg",
                                     bufs=3, name=f"wg_t{k}")
                for ii in range(IT):
                    nc.sync.dma_start(out=wg_t[k][:, ii], in_=wg_d[k][:, ii])
                wu_t[k] = wpool.tile([P128, IT, HT, P128], F16, tag="wu",
                                     bufs=3, name=f"wu_t{k}")
                for ii in range(IT):
                    nc.sync.dma_start(out=wu_t[k][:, ii], in_=wu_d[k][:, ii])
                wd_t[k] = wpool.tile([P128, IT, H], F16, tag="wd", bufs=3,
                                     name=f"wd_t{k}")
                nc.sync.dma_start(out=wd_t[k][:], in_=wd_d[k][:])

            def _slot_compute(k):
                acts = [apool.tile([P128, P[k]], F16, tag="acts", bufs=8,
                                   name=f"acts{ii}") for ii in range(IT)]
                for (mo, mw) in _chunks(P[k]):
                    for ii in range(IT):
                        h1 = ps.tile([P128, 512], F32, tag="h1")
                        h2 = ps.tile([P128, 512], F32, tag="h2")
                        for h in range(HT):
                            nc.tensor.matmul(
                                h1[:, :mw],
                                lhsT=wg_t[k][:, ii, h, :],
                                rhs=xe_t[k][:, h, mo:mo + mw],
                                start=(h == 0), stop=(h == HT - 1))
                        for h in range(HT):
                            nc.tensor.matmul(
                                h2[:, :mw],
                                lhsT=wu_t[k][:, ii, h, :],
                                rhs=xe_t[k][:, h, mo:mo + mw],
                                start=(h == 0), stop=(h == HT - 1))
                        sl = stpool.tile([P128, 512], F32, tag="silu", bufs=3,
                                         name="sl")
                        nc.scalar.activation(sl[:, :mw], h1[:, :mw], AF.Silu)
                        nc.vector.tensor_mul(acts[ii][:, mo:mo + mw],
                                             sl[:, :mw], h2[:, :mw])

                for cc in range(nck[k]):
                    pp = min(P128, P[k] - cc * P128)
                    ost = stpool.tile([P128, H], F16, tag="ost", bufs=4,
                                      name="ost")
                    for hh in range(2):
                        dps = ps.tile([P128, H // 2], F32, tag="dps", bufs=4)
                        for ii in range(IT):
                            nc.tensor.matmul(
                                dps[:pp, :],
                                lhsT=acts[ii][:, cc * P128:cc * P128 + pp],
                                rhs=wd_t[k][:, ii,
                                            hh * (H // 2):(hh + 1) * (H // 2)],
                                start=(ii == 0), stop=(ii == IT - 1))
                        nc.vector.tensor_scalar_mul(
                            ost[:pp, hh * (H // 2):(hh + 1) * (H // 2)],
                            dps[:pp, :], sc_t[k][:pp, cc:cc + 1])
                    nc.gpsimd.dma_start(
                        out=ro_d[k][cc * P128:cc * P128 + pp, :],
                        in_=ost[:pp, :])

            # smallest slot's inputs first: tiny DMA prefix starts the PE early
            _load_slot(S - 1)

            # shared expert inputs
            xt_t = cpool.tile([P128, HT, T], F16, tag="xt")
            nc.sync.dma_start(out=xt_t[:], in_=xt_d[:])
            wsg_t = cpool.tile([P128, IST, HT, P128], F16, tag="wsg")
            for ii in range(IST):
                nc.sync.dma_start(out=wsg_t[:, ii], in_=wsg_d[:, ii])
            wsu_t = cpool.tile([P128, IST, HT, P128], F16, tag="wsu")
            for ii in range(IST):
                nc.sync.dma_start(out=wsu_t[:, ii], in_=wsu_d[:, ii])
            wsd_t = cpool.tile([P128, IST, H], F16, tag="wsd")
            nc.sync.dma_start(out=wsd_t[:], in_=wsd_d[:])

            _load_slot(0)
            _load_slot(1)

            # ---- compute: smallest slot, then shared, then slots desc ----
            _slot_compute(S - 1)

            # shared expert (intermediate shard ISH=256)
            acts_s = [apool.tile([P128, T], F16, tag="acts_s", bufs=2,
                                 name=f"acts_s{ii}") for ii in range(IST)]
            for (mo, mw) in _chunks(T):
                for ii in range(IST):
                    h1 = ps.tile([P128, 512], F32, tag="h1")
                    h2 = ps.tile([P128, 512], F32, tag="h2")
                    for h in range(HT):
                        nc.tensor.matmul(
                            h1[:, :mw], lhsT=wsg_t[:, ii, h, :],
                            rhs=xt_t[:, h, mo:mo + mw],
                            start=(h == 0), stop=(h == HT - 1))
                    for h in range(HT):
                        nc.tensor.matmul(
                            h2[:, :mw], lhsT=wsu_t[:, ii, h, :],
                            rhs=xt_t[:, h, mo:mo + mw],
                            start=(h == 0), stop=(h == HT - 1))
                    sl = stpool.tile([P128, 512], F32, tag="silu", bufs=3,
                                     name="sl")
                    nc.scalar.activation(sl[:, :mw], h1[:, :mw], AF.Silu)
                    nc.vector.tensor_mul(acts_s[ii][:, mo:mo + mw],
                                         sl[:, :mw], h2[:, :mw])

            for cc in range(T // P128):
                ost = stpool.tile([P128, H], F16, tag="ost", bufs=4,
                                  name="ost")
                for hh in range(2):
                    dps = ps.tile([P128, H // 2], F32, tag="dps", bufs=4)
                    for ii in range(IST):
                        nc.tensor.matmul(
                            dps[:],
                            lhsT=acts_s[ii][:, cc * P128:(cc + 1) * P128],
                            rhs=wsd_t[:, ii, hh * (H // 2):(hh + 1) * (H // 2)],
                            start=(ii == 0), stop=(ii == IST - 1))
                    nc.vector.tensor_copy(
                        ost[:, hh * (H // 2):(hh + 1) * (H // 2)], dps[:])
                nc.gpsimd.dma_start(
                    out=so_d[cc * P128:(cc + 1) * P128, :], in_=ost[:])

            # remaining slots, descending capacity
            for k in range(S - 1):
                if k + 2 < S - 1:
                    _load_slot(k + 2)
                _slot_compute(k)

    nc.compile()
    return nc


def _prepare(inputs):
    """Host-side dispatch prep: returns (in_maps, P, slot_expert, tok_lists)."""
    x = np.ascontiguousarray(inputs["hidden_states"], dtype=np.float32)
    gate_w = np.asarray(inputs["gate_w"], dtype=np.float32)
    e_bias = np.asarray(inputs["e_bias"], dtype=np.float32)
    w_gate = np.asarray(inputs["w_gate"], dtype=np.float32)
    w_up = np.asarray(inputs["w_up"], dtype=np.float32)
    w_down = np.asarray(inputs["w_down"], dtype=np.float32)
    ws_gate = np.asarray(inputs["ws_gate"], dtype=np.float32)
    ws_up = np.asarray(inputs["ws_up"], dtype=np.float32)
    ws_down = np.asarray(inputs["ws_down"], dtype=np.float32)

    # ---- dispatch metadata ----
    emask, combine = _host_routing(x, gate_w, e_bias)
    counts = emask.sum(0).astype(np.int64)
    order = np.argsort(-counts, kind="stable")
    tok_lists = [np.nonzero(emask[:, e])[0] for e in range(E)]
    # slot k of every core serves tier k (descending size); capacity = exact
    # max count of the tier (moving dims need no padding in fp16)
    tiers = [[int(order[k * NCORES + c]) for c in range(NCORES)]
             for k in range(S)]
    P = [max(8, max(int(counts[e]) for e in tier)) for tier in tiers]

    x16 = x.astype(np.float16)
    xt = _pmajor(np.ascontiguousarray(x16.T))          # [128, HT, T]
    in_maps = []
    slot_expert = np.zeros((NCORES, S), dtype=np.int64)
    wg16 = w_gate.astype(np.float16)
    wu16 = w_up.astype(np.float16)
    wd16 = w_down.astype(np.float16)
    for c in range(NCORES):
        m = {"xt": xt,
             "wsg": _iimajor(ws_gate[:, c * ISH:(c + 1) * ISH].astype(np.float16)),
             "wsu": _iimajor(ws_up[:, c * ISH:(c + 1) * ISH].astype(np.float16)),
             "wsd": _pmajor(ws_down[c * ISH:(c + 1) * ISH, :].astype(np.float16))}
        for k in range(S):
            e = tiers[k][c]
            slot_expert[c, k] = e
            toks = tok_lists[e]
            n = len(toks)
            n_c = (P[k] + P128 - 1) // P128
            xe = np.zeros((P128, HT, P[k]), dtype=np.float16)
            xe[:, :, :n] = _pmajor(np.ascontiguousarray(x16[toks].T))
            scv = np.zeros(n_c * P128, dtype=np.float32)
            scv[:n] = combine[toks, e]
            m[f"xe{k}"] = xe
            m[f"wg{k}"] = _iimajor(wg16[e])
            m[f"wu{k}"] = _iimajor(wu16[e])
            m[f"wd{k}"] = _pmajor(wd16[e])
            m[f"sc{k}"] = np.ascontiguousarray(scv.reshape(n_c, P128).T)
        in_maps.append(m)

    return in_maps, P, slot_expert, tok_lists


def _recombine(results, slot_expert, tok_lists):
    out = np.zeros((T, H), dtype=np.float32)
    for c in range(NCORES):
        out += results[c]["so"].astype(np.float32)
    for c in range(NCORES):
        for k in range(S):
            e = slot_expert[c, k]
            toks = tok_lists[e]
            out[toks] += results[c][f"ro{k}"][:len(toks)].astype(np.float32)
    return out


def kernel(**inputs):
    global LAST_RESULTS
    in_maps, P, slot_expert, tok_lists = _prepare(inputs)
    nc = _build_program(P)
    trace = bool(int(os.environ.get("KERNEL_TRACE", "0")))
    if trace:
        trace = _install_ntff_hook()
    LAST_RESULTS = run_bass_kernel_spmd(
        nc, in_maps, list(range(NCORES)), trace=trace)
    results = LAST_RESULTS.results
    return _recombine(results, slot_expert, tok_lists)
